# revision 5
# baseline (speedup 1.0000x reference)
"""BraggNN Trainium2 kernel — program builder + host prep.

Layouts (per core, BC samples, tiles of NBT=64):
  xcol  [10, BC*81] fp16   im2col of conv1 input + ones row (host-built)
  out   [2, BC]     fp32   final FC output (host transposes)

Math folding (host):
  out1 = W1a^T xcol            (W1a [10,64], rows 0:9 = w1 taps, row 9 = b1)
  Z^T  = (M^T xcol_b)^T xcol_b (M = Wp_aug @ Wt_aug^T  [10,10])
  g    = xcol_b^T Wg_aug       (Wg_aug [10,32] folded wg@conv1 + bias)
  yu^T_aug = g_aug^T E         (g_aug [81,33] col 32 = ones -> row 32 = denom)
  denom_bcast = ones_lhs^T E   (ones_lhs [81,33])
  yn = yu * recip(denom)       (row 32 becomes ones)
  o  = [wo^T;bo]^T yn_aug + W1a^T xcol   (residual + bias via 2 accumulating MMs)
  x2 = lrelu(o); conv2/conv3 via dx-shifted partition-packed tap matmuls
  fc chain with ones-row bias folding
"""
import sys

sys.path.insert(0, "/opt/trn_rl_repo")

import numpy as np

import concourse.bass as bass
import concourse.mybir as mybir
import concourse.tile as tile
from concourse import bacc

F16 = mybir.dt.float16
F32 = mybir.dt.float32
ALU = mybir.AluOpType
ACTF = mybir.ActivationFunctionType

NBT = 64  # samples per tile iteration


def groups6(n):
    out = []
    b0 = 0
    while b0 < n:
        out.append((b0, min(6, n - b0)))
        b0 += 6
    return out


def host_prep(inputs, n_cores=8):
    """Fold weights, build xcol. Returns (per_core_in_maps, const_map)."""
    f = {k: np.asarray(v, dtype=np.float32) for k, v in inputs.items()}
    w1 = f["w1"].reshape(64, 9)          # [o, tap]
    b1 = f["b1"]
    w1a = np.concatenate([w1.T, b1[None, :]], 0)                    # [10, 64]
    wt_e = f["wt"] @ w1                                             # [32, 9]
    bt_e = f["wt"] @ b1 + f["bt"]
    wt_aug = np.concatenate([wt_e.T, bt_e[None, :]], 0)             # [10, 32]
    wp_e = f["wp"] @ w1
    bp_e = f["wp"] @ b1 + f["bp"]
    wp_aug = np.concatenate([wp_e.T, bp_e[None, :]], 0)             # [10, 32]
    m_pt = wp_aug @ wt_aug.T                                        # [10, 10]
    wg_e = f["wg"] @ w1
    bg_e = f["wg"] @ b1 + f["bg"]
    wg_aug = np.concatenate([wg_e.T, bg_e[None, :]], 0)             # [10, 32]
    w_ab = np.concatenate([f["wo"].T, f["bo"][None, :]], 0)         # [33, 64]

    w2 = f["w2"]  # [32, 64, 3, 3]
    w2t0 = np.zeros((128, 3, 32), np.float32)
    w2t1 = np.zeros((65, 3, 32), np.float32)
    for dy in range(3):
        for dx in range(2):
            w2t0[dx * 64:(dx + 1) * 64, dy, :] = w2[:, :, dy, dx].T
        w2t1[0:64, dy, :] = w2[:, :, dy, 2].T
    w2t1[64, 0, :] = f["b2"]

    w3 = f["w3"]  # [8, 32, 3, 3]
    w3r = np.zeros((97, 3, 8), np.float32)
    for dy in range(3):
        for dx in range(3):
            w3r[dx * 32:(dx + 1) * 32, dy, :] = w3[:, :, dy, dx].T
    w3r[96, 0, :] = f["b3"]

    fw1a = np.concatenate([f["fw1"][:, 0:100].T, f["fb1"][None, :]], 0)  # [101,64]
    fw1b = f["fw1"][:, 100:200].T                                        # [100,64]
    fw2a = np.concatenate([f["fw2"].T, f["fb2"][None, :]], 0)            # [65,32]
    fw3a = np.concatenate([f["fw3"].T, f["fb3"][None, :]], 0)            # [33,16]
    fw4a = np.concatenate([f["fw4"].T, f["fb4"][None, :]], 0)            # [17,8]
    fwoa = np.concatenate([f["fwo"].T, f["fbo"][None, :]], 0)            # [9,2]

    x = f["x"].reshape(-1, 11, 11)
    B = x.shape[0]
    win = np.lib.stride_tricks.sliding_window_view(x, (3, 3), axis=(1, 2))
    # win [B, 9, 9, 3, 3] -> [tap(3,3), B, pos(9,9)]
    xcol = win.transpose(3, 4, 0, 1, 2).reshape(9, B, 81)
    xcol = np.concatenate([xcol, np.ones((1, B, 81), np.float32)], 0)  # [10,B,81]
    xcol = xcol.astype(np.float16)

    const = {
        "m_pt": m_pt, "wg_aug": wg_aug, "w_ab": w_ab, "w_o1": w1a,
        "w2t0": w2t0, "w2t1": w2t1, "w3r": w3r,
        "fw1a": fw1a, "fw1b": fw1b, "fw2a": fw2a, "fw3a": fw3a,
        "fw4a": fw4a, "fwoa": fwoa,
    }
    const = {k: v.astype(np.float16) for k, v in const.items()}

    bc = B // n_cores
    in_maps = []
    for c in range(n_cores):
        m = dict(const)
        m["xcol"] = np.ascontiguousarray(
            xcol[:, c * bc:(c + 1) * bc, :].reshape(10, bc * 81))
        in_maps.append(m)
    return in_maps, bc


def build_program(BC):
    """Build the Bass program for one core handling BC samples."""
    assert BC % NBT == 0
    n_tiles = BC // NBT
    nc = bacc.Bacc("TRN2", target_bir_lowering=False, debug=False,
                   enable_asserts=False)

    xcol_d = nc.dram_tensor("xcol", [10, BC * 81], F16, kind="ExternalInput").ap()
    out_d = nc.dram_tensor("out", [2, BC], F32, kind="ExternalOutput").ap()
    cshape = dict(m_pt=[10, 10], wg_aug=[10, 32], w_ab=[33, 64], w_o1=[10, 64],
                  w2t0=[128, 3, 32], w2t1=[65, 3, 32], w3r=[97, 3, 8],
                  fw1a=[101, 64], fw1b=[100, 64], fw2a=[65, 32],
                  fw3a=[33, 16], fw4a=[17, 8], fwoa=[9, 2])
    cd = {k: nc.dram_tensor(k, v, F16, kind="ExternalInput").ap()
          for k, v in cshape.items()}

    import contextlib
    with tile.TileContext(nc) as tc, contextlib.ExitStack() as ctx:
        cp = ctx.enter_context(tc.tile_pool(name="consts", bufs=1))
        C = {}
        for k, shp in cshape.items():
            C[k] = cp.tile(shp, F16, tag=k, name=k)
            nc.sync.dma_start(out=C[k], in_=cd[k])
        ones_lhs = cp.tile([81, 33], F16, tag="ones_lhs")
        nc.vector.memset(ones_lhs, 1.0)
        ones_row = cp.tile([1, NBT * 81], F16, tag="ones_row")
        nc.vector.memset(ones_row, 1.0)
        ones_col = cp.tile([81, 6], F16, tag="ones_col")
        nc.vector.memset(ones_col, 1.0)

        p_xc = ctx.enter_context(tc.tile_pool(name="p_xc", bufs=2))
        p_d = ctx.enter_context(tc.tile_pool(name="p_d", bufs=3))
        p_e = ctx.enter_context(tc.tile_pool(name="p_e", bufs=3))
        p_g = ctx.enter_context(tc.tile_pool(name="p_g", bufs=3))
        p_r = ctx.enter_context(tc.tile_pool(name="p_r", bufs=3))
        p_yn = ctx.enter_context(tc.tile_pool(name="p_yn", bufs=3))
        p_x2 = ctx.enter_context(tc.tile_pool(name="p_x2", bufs=2))
        p_x3 = ctx.enter_context(tc.tile_pool(name="p_x3", bufs=2))
        p_x4 = ctx.enter_context(tc.tile_pool(name="p_x4", bufs=2))
        p_h = ctx.enter_context(tc.tile_pool(name="p_h", bufs=2))
        p_o = ctx.enter_context(tc.tile_pool(name="p_o", bufs=2))

        q_c1 = ctx.enter_context(tc.tile_pool(name="q_c1", bufs=1, space="PSUM"))
        q_zt = ctx.enter_context(tc.tile_pool(name="q_zt", bufs=1, space="PSUM"))
        q_g = ctx.enter_context(tc.tile_pool(name="q_g", bufs=1, space="PSUM"))
        q_yu = ctx.enter_context(tc.tile_pool(name="q_yu", bufs=1, space="PSUM"))
        q_dn = ctx.enter_context(tc.tile_pool(name="q_dn", bufs=1, space="PSUM"))
        q_or = ctx.enter_context(tc.tile_pool(name="q_or", bufs=1, space="PSUM"))
        q_cf = ctx.enter_context(tc.tile_pool(name="q_cf", bufs=2, space="PSUM"))

        def lrelu(dst, src):
            # parametric relu, alpha=0.01; same ACT table set as Exp
            nc.scalar.activation(dst, src, ACTF.Prelu,
                                 bias=0.0, scale=1.0, alpha=0.01)

        for it in range(n_tiles):
            base = it * NBT
            xc = p_xc.tile([10, NBT * 81], F16, tag="xc")
            nc.sync.dma_start(out=xc, in_=xcol_d[:, base * 81:(base + NBT) * 81])
            x2t0 = p_x2.tile([128, NBT, 9, 9], F16, tag="x2t0")
            x2t1 = p_x2.tile([65, NBT, 9, 9], F16, tag="x2t1")
            nc.sync.dma_start(out=x2t1[64:65, :, :, :],
                              in_=ones_row[0:1, :].rearrange(
                                  "p (b i j) -> p b i j", i=9, j=9))
            x3row = p_x3.tile([97, NBT, 7, 7], F16, tag="x3row")
            nc.sync.dma_start(out=x3row[96:97, :, :, :],
                              in_=ones_row[0:1, 0:NBT * 49].rearrange(
                                  "p (b i j) -> p b i j", i=7, j=7))
            x3o = p_x3.tile([8, 25, NBT], F16, tag="x3o")

            for (b0, gn) in groups6(NBT):
                w = gn * 81
                xcv = xc[:, b0 * 81:b0 * 81 + w]
                cps = q_c1.tile([10, 486], F32, tag="c1")
                nc.tensor.matmul(cps[:, 0:w], C["m_pt"], xcv,
                                 start=True, stop=True)
                D = p_d.tile([10, 486], F16, tag="D")
                nc.vector.tensor_copy(D[:, 0:w], cps[:, 0:w])

                gps = q_g.tile([81, 192], F32, tag="gps")
                for j in range(gn):
                    nc.tensor.matmul(
                        gps[:, j * 32:(j + 1) * 32],
                        xc[:, (b0 + j) * 81:(b0 + j + 1) * 81],
                        C["wg_aug"], start=True, stop=True)
                gsb = p_g.tile([81, 6, 33], F16, tag="gsb")
                nc.sync.dma_start(out=gsb[:, 0:gn, 32:33],
                                  in_=ones_col[:, 0:gn])
                nc.vector.tensor_copy(
                    gsb[:, 0:gn, 0:32],
                    gps[:, 0:gn * 32].rearrange("p (b i) -> p b i", i=32))

                ztp = q_zt.tile([81, 486], F32, tag="zt")
                for j in range(gn):
                    nc.tensor.matmul(
                        ztp[:, j * 81:(j + 1) * 81],
                        D[:, j * 81:(j + 1) * 81],
                        xc[:, (b0 + j) * 81:(b0 + j + 1) * 81],
                        start=True, stop=True)
                esb = p_e.tile([81, 486], F16, tag="E")
                nc.scalar.activation(esb[:, 0:w], ztp[:, 0:w], ACTF.Exp)

                yup = q_yu.tile([33, 486], F32, tag="yu")
                dnp = q_dn.tile([33, 486], F32, tag="dn")
                nc.tensor.matmul(dnp[:, 0:w], ones_lhs, esb[:, 0:w],
                                 start=True, stop=True)
                for j in range(gn):
                    nc.tensor.matmul(
                        yup[:, j * 81:(j + 1) * 81], gsb[:, j, :],
                        esb[:, j * 81:(j + 1) * 81], start=True, stop=True)
                r = p_r.tile([33, 486], F32, tag="r")
                nc.vector.reciprocal_approx_fast(out=r[:, 0:w], in_=dnp[:, 0:w])
                yn = p_yn.tile([33, 486], F16, tag="yn")
                nc.vector.tensor_mul(yn[:, 0:w], yup[:, 0:w], r[:, 0:w])

                orp = q_or.tile([64, 486], F32, tag="or")
                nc.tensor.matmul(orp[:, 0:w], C["w_ab"], yn[:, 0:w],
                                 start=True, stop=False)
                nc.tensor.matmul(orp[:, 0:w], C["w_o1"], xcv,
                                 start=False, stop=True)
                lrelu(x2t0[0:64, b0:b0 + gn, :, :], orp[:, 0:w])
                nc.sync.dma_start(out=x2t0[64:128, b0:b0 + gn, :, 0:8],
                                  in_=x2t0[0:64, b0:b0 + gn, :, 1:9])
                nc.sync.dma_start(out=x2t1[0:64, b0:b0 + gn, :, 0:7],
                                  in_=x2t0[0:64, b0:b0 + gn, :, 2:9])

            for q in range(8):  # conv2: groups of 8 samples
                b0 = q * 8
                c2 = q_cf.tile([32, 8, 7, 7], F32, tag="cf")
                for dy in range(3):
                    nc.tensor.matmul(c2, C["w2t0"][:, dy, :],
                                     x2t0[:, b0:b0 + 8, dy:dy + 7, 0:7],
                                     start=(dy == 0), stop=False)
                    nc.tensor.matmul(c2, C["w2t1"][:, dy, :],
                                     x2t1[:, b0:b0 + 8, dy:dy + 7, 0:7],
                                     start=False, stop=(dy == 2))
                lrelu(x3row[0:32, b0:b0 + 8, :, :], c2[:, :, :, :])
                nc.sync.dma_start(out=x3row[32:64, b0:b0 + 8, :, 0:6],
                                  in_=x3row[0:32, b0:b0 + 8, :, 1:7])
                nc.sync.dma_start(out=x3row[64:96, b0:b0 + 8, :, 0:5],
                                  in_=x3row[0:32, b0:b0 + 8, :, 2:7])

            for q in range(4):  # conv3: groups of 16 samples
                b0 = q * 16
                c3 = q_cf.tile([8, 5, 5, 16], F32, tag="cf")
                for dy in range(3):
                    rhs = x3row[:, b0:b0 + 16, dy:dy + 5, 0:5].rearrange(
                        "k b i j -> k i j b")
                    nc.tensor.matmul(c3, C["w3r"][:, dy, :], rhs,
                                     start=(dy == 0), stop=(dy == 2))
                lrelu(x3o[:, :, b0:b0 + 16],
                      c3[:, :, :, :].rearrange("k i j b -> k (i j) b"))

            x4a = p_x4.tile([101, NBT], F16, tag="x4a")
            x4b = p_x4.tile([100, NBT], F16, tag="x4b")
            nc.sync.dma_start(out=x4a[0:100, :], in_=x3o[0:4, :, :])
            nc.sync.dma_start(out=x4a[100:101, :], in_=ones_row[0:1, 0:NBT])
            nc.sync.dma_start(out=x4b[0:100, :], in_=x3o[4:8, :, :])

            f1 = q_cf.tile([64, NBT], F32, tag="cf")
            nc.tensor.matmul(f1, C["fw1a"], x4a, start=True, stop=False)
            nc.tensor.matmul(f1, C["fw1b"], x4b, start=False, stop=True)
            h1 = p_h.tile([65, NBT], F16, tag="h1")
            lrelu(h1[0:64, :], f1[:, :])
            nc.sync.dma_start(out=h1[64:65, :], in_=ones_row[0:1, 0:NBT])

            f2 = q_cf.tile([32, NBT], F32, tag="cf")
            nc.tensor.matmul(f2, C["fw2a"], h1, start=True, stop=True)
            h2 = p_h.tile([33, NBT], F16, tag="h2")
            lrelu(h2[0:32, :], f2[:, :])
            nc.sync.dma_start(out=h2[32:33, :], in_=ones_row[0:1, 0:NBT])

            f3 = q_cf.tile([16, NBT], F32, tag="cf")
            nc.tensor.matmul(f3, C["fw3a"], h2, start=True, stop=True)
            h3 = p_h.tile([17, NBT], F16, tag="h3")
            lrelu(h3[0:16, :], f3[:, :])
            nc.sync.dma_start(out=h3[16:17, :], in_=ones_row[0:1, 0:NBT])

            f4 = q_cf.tile([8, NBT], F32, tag="cf")
            nc.tensor.matmul(f4, C["fw4a"], h3, start=True, stop=True)
            h4 = p_h.tile([9, NBT], F16, tag="h4")
            lrelu(h4[0:8, :], f4[:, :])
            nc.sync.dma_start(out=h4[8:9, :], in_=ones_row[0:1, 0:NBT])

            fo = q_cf.tile([2, NBT], F32, tag="cf")
            nc.tensor.matmul(fo, C["fwoa"], h4, start=True, stop=True)
            outt = p_o.tile([2, NBT], F32, tag="out")
            nc.scalar.copy(outt, fo[:, :])
            nc.sync.dma_start(out=out_d[:, base:base + NBT], in_=outt)

    nc.compile()
    return nc


_CACHE = {}


def _get_program(bc):
    if bc not in _CACHE:
        _CACHE[bc] = build_program(bc)
    return _CACHE[bc]


def kernel(**inputs):
    """Full-input entry: shard batch over 8 cores, run SPMD, gather."""
    from concourse.bass_utils import run_bass_kernel_spmd

    n_cores = 8
    in_maps, bc = host_prep(inputs, n_cores=n_cores)
    nc = _get_program(bc)
    res = run_bass_kernel_spmd(nc, in_maps, list(range(n_cores)))
    outs = [res.results[i]["out"] for i in range(n_cores)]
    return np.ascontiguousarray(
        np.concatenate(outs, axis=1).T.astype(np.float32))
